# revision 2
# baseline (speedup 1.0000x reference)
"""Trainium2 single-invocation Bass kernel for nn_HSauteUnit.

The entire 2-layer x 16-turn speaker-memory recurrence runs in ONE
8-core SPMD NEFF invocation (the previous kernel used 7 invocations at
~81ms RPC floor each).  Structure:

- Runtime schedule from speaker_ids: per layer, turns fall into
  dependency "waves" (W(b) = 1 + max_{j<=prev(b)} W(j)); each wave is an
  encoder slot where up to 8 turns run data-parallel, one per core
  (idle cores duplicate a real turn; results ignored).
- Layer-phased: all layer-0 slots+gates, then layer-1 (keeps one
  layer's gate weights SBUF-resident).  Turn b is pinned to the same
  core in both phases so the layer-0 output stays core-local (DRAM
  scratch).
- Speaker-memory gates run REPLICATED on all 8 cores (identical math,
  so no state broadcast).  The only cross-core traffic is a small
  AllGather of per-turn utterance vectors after each slot.
- Gate matvecs are batched per-slot where linearity allows
  (U1 = u @ Wg1_top, AT = (u @ Wattn)^T for all of the slot's turns).

Matmuls in bf16 with f32 accumulation; LN / softmax / speaker state in
f32.  Zero biases and unit LN scales are folded (checked on the host,
full-host numpy fallback otherwise).
"""

import os
import numpy as np

B, T, D, H, F, L, V, P, S, U = 16, 512, 768, 12, 3072, 2, 32000, 512, 8, 2048
HD = D // H
N_CORES = 8
DT = D // 128     # 6
MT = T // 128     # 4
UT = U // 128     # 16
NHG = 2           # head groups
HPG = H // NHG    # 6 heads per group

LAST_EXEC_NS = -1


# ------------------------------------------------------------------ schedule
def _schedule(sids):
    """Waves per layer + gate placement + turn->core assignment."""
    prev = [-1] * B
    last = {}
    for b in range(B):
        s = int(sids[b])
        prev[b] = last.get(s, -1)
        last[s] = b
    Wv = [0] * B
    for b in range(B):
        if prev[b] >= 0:
            Wv[b] = 1 + max(Wv[j] for j in range(prev[b] + 1))
    nw = max(Wv) + 1
    waves = [[b for b in range(B) if Wv[b] == k] for k in range(nw)]
    groups, placed = [], 0
    for k in range(nw):
        grp = []
        while placed < B and Wv[placed] <= k:
            grp.append(placed)
            placed += 1
        groups.append(grp)
    assert placed == B
    # turn -> (wave k, core i);  dummy cores replicate wave[0]
    assign = {}
    for k, wave in enumerate(waves):
        for i, b in enumerate(wave):
            assert i < N_CORES
            assign[b] = (k, i)
    return waves, groups, assign, prev


# ------------------------------------------------------------------- builder
def _build(sids, waves, groups):
    from contextlib import ExitStack
    import concourse.bass as bass
    import concourse.tile as tile
    from concourse import bacc, mybir
    from concourse.masks import make_identity

    f32 = mybir.dt.float32
    bf16 = mybir.dt.bfloat16
    AF = mybir.ActivationFunctionType
    NW = len(waves)

    nc = bacc.Bacc("TRN2", target_bir_lowering=False, debug=False,
                   num_devices=N_CORES)

    # ---- DRAM I/O
    xemb_d = nc.dram_tensor("xemb", [NW * T, D], bf16, kind="ExternalInput")
    mfs_d = nc.dram_tensor("mfs", [128, NW * MT], bf16, kind="ExternalInput")
    icnt_d = nc.dram_tensor("icnt", [1, NW], f32, kind="ExternalInput")
    onehc_d = nc.dram_tensor("onehc", [S, NW], bf16, kind="ExternalInput")
    onehr_d = nc.dram_tensor("onehr", [1, B * S], bf16, kind="ExternalInput")
    wd = {}
    for l in range(L):
        wd[f"wqk{l}"] = nc.dram_tensor(f"wqk{l}", [D, 2 * D], bf16, kind="ExternalInput")
        wd[f"wv{l}"] = nc.dram_tensor(f"wv{l}", [D, D], bf16, kind="ExternalInput")
        wd[f"wo{l}"] = nc.dram_tensor(f"wo{l}", [D, D], bf16, kind="ExternalInput")
        wd[f"w1{l}"] = nc.dram_tensor(f"w1{l}", [D, F], bf16, kind="ExternalInput")
        wd[f"w2{l}"] = nc.dram_tensor(f"w2{l}", [F, D], bf16, kind="ExternalInput")
        wd[f"wg1{l}"] = nc.dram_tensor(f"wg1{l}", [3 * D, U], bf16, kind="ExternalInput")
        wd[f"wg2{l}"] = nc.dram_tensor(f"wg2{l}", [U, D], bf16, kind="ExternalInput")
        wd[f"wattn{l}"] = nc.dram_tensor(f"wattn{l}", [D, D], bf16, kind="ExternalInput")
        wd[f"wproj{l}"] = nc.dram_tensor(f"wproj{l}", [D, D], bf16, kind="ExternalInput")
    xout_d = nc.dram_tensor("xout", [NW * T, D], bf16, kind="ExternalOutput")

    def tiled(h, p=128):
        return h.ap().rearrange("(a p) n -> p a n", p=p)

    with tile.TileContext(nc) as tc, ExitStack() as ctx:
        const = ctx.enter_context(tc.tile_pool(name="const", bufs=1))
        wres = ctx.enter_context(tc.tile_pool(name="wres", bufs=1))
        wstr = ctx.enter_context(tc.tile_pool(name="wstr", bufs=1))
        acts = ctx.enter_context(tc.tile_pool(name="acts", bufs=1))
        gst = ctx.enter_context(tc.tile_pool(name="gst", bufs=1))
        sm = ctx.enter_context(tc.tile_pool(name="sm", bufs=2))
        dram = ctx.enter_context(tc.tile_pool(name="dram", bufs=1, space="DRAM"))
        pmm = ctx.enter_context(tc.tile_pool(name="pmm", bufs=5, space="PSUM"))
        ptp = ctx.enter_context(tc.tile_pool(name="ptp", bufs=3, space="PSUM"))

        ident = const.tile([128, 128], bf16)
        make_identity(nc, ident[:])
        ones64 = const.tile([1, 64], f32)
        nc.vector.memset(ones64[:], 1.0)
        ones128 = const.tile([1, 128], bf16)
        nc.vector.memset(ones128[:], 1.0)

        mfs = const.tile([128, NW, MT], bf16, name="mfs")
        nc.sync.dma_start(out=mfs[:],
                          in_=mfs_d.ap().rearrange("p (s m) -> p s m", m=MT))
        icnt = const.tile([1, NW], f32, name="icnt")
        nc.sync.dma_start(out=icnt[:], in_=icnt_d.ap())
        onehc = const.tile([S, NW], bf16, name="onehc")
        nc.sync.dma_start(out=onehc[:], in_=onehc_d.ap())
        onehr = const.tile([1, B, S], bf16, name="onehr")
        nc.sync.dma_start(out=onehr[:],
                          in_=onehr_d.ap().rearrange("p (b s) -> p b s", s=S))

        # DRAM scratch + bounce buffers
        x0sc = [dram.tile([128, MT, D], bf16, tag=f"x0_{k}", name=f"x0_{k}")
                for k in range(NW)]
        agin = [dram.tile([1, D], f32, tag=f"agi{j}", name=f"agi{j}")
                for j in range(2 * NW)]
        agout = [dram.tile([N_CORES, D], f32, tag=f"ago{j}", name=f"ago{j}")
                 for j in range(2 * NW)]

        # persistent gate state / per-slot u-projection tiles
        h_spk = gst.tile([S, D], f32, name="h_spk")
        h_bf = gst.tile([S, D], bf16, name="h_bf")
        hT_bf = gst.tile([128, DT, S], bf16, name="hT_bf")
        U1slot = [dram.tile([S, U], bf16, tag=f"u1s{k}", name=f"u1s{k}")
                  for k in range(NW)]
        ATslot = [gst.tile([128, DT, N_CORES], bf16, tag=f"ats{k}",
                           name=f"ats{k}") for k in range(NW)]
        assign = {}
        for k, wave in enumerate(waves):
            for i, b in enumerate(wave):
                assign[b] = (k, i)

        def slot_code(phase, k, wave, slot_j):
            l = phase
            # -- xin = src + (h_spk[sid] @ Wproj) broadcast row
            hrowT = sm.tile([128, DT, 1], bf16, tag="hrowT",
                            name=f"hrowT{slot_j}")
            for kk in range(DT):
                ps = pmm.tile([128, 1], f32, tag="mm", padded_shape=[128, 512],
                              name=f"hr{slot_j}_{kk}")
                nc.tensor.matmul(ps[:], h_bf[:, 128 * kk:128 * (kk + 1)],
                                 onehc[:, k:k + 1], start=True, stop=True)
                nc.vector.tensor_copy(hrowT[:, kk, :], ps[:])
            c_bf = sm.tile([1, D], bf16, tag="c_bf", name=f"cbf{slot_j}", bufs=1)
            for n in range(2):
                wproj_s = wstr.tile([128, DT, 384], bf16, tag="wproj",
                                    name=f"wpj{slot_j}_{n}", bufs=1)
                nc.sync.dma_start(out=wproj_s[:],
                                  in_=tiled(wd[f"wproj{l}"])[:, :, 384 * n:384 * (n + 1)])
                ps = pmm.tile([1, 384], f32, tag="mm", padded_shape=[128, 512],
                              name=f"c{slot_j}_{n}")
                for kk in range(DT):
                    nc.tensor.matmul(ps[:], hrowT[:, kk, :], wproj_s[:, kk, :],
                                     start=(kk == 0), stop=(kk == DT - 1))
                nc.vector.tensor_copy(c_bf[:, 384 * n:384 * (n + 1)], ps[:])
            xin_s = acts.tile([128, MT, D], bf16, name=f"xin{slot_j}",
                              tag="xin", bufs=2)
            src = tiled(xemb_d)[:, 4 * k:4 * k + 4, :] if phase == 0 \
                else x0sc[k][:]
            nc.sync.dma_start(out=xin_s[:], in_=src)
            for n in range(2):
                psr = pmm.tile([128, 384], f32, tag="mm",
                               padded_shape=[128, 512], name=f"rep{slot_j}_{n}")
                nc.tensor.matmul(psr[:], ones128[:],
                                 c_bf[:, 384 * n:384 * (n + 1)],
                                 start=True, stop=True)
                for m in range(MT):
                    nc.vector.tensor_add(
                        xin_s[:, m, 384 * n:384 * (n + 1)],
                        xin_s[:, m, 384 * n:384 * (n + 1)], psr[:])
            # -- xt (feature-major)
            xt = acts.tile([128, DT, T], bf16, name=f"xt{slot_j}", tag="xtsh", bufs=1)
            for dt in range(DT):
                for m in range(MT):
                    ps = ptp.tile([128, 128], bf16, tag="tp",
                                  padded_shape=[128, 1024],
                                  name=f"xtt{slot_j}_{dt}_{m}")
                    nc.tensor.transpose(ps[:],
                                        xin_s[:, m, 128 * dt:128 * (dt + 1)],
                                        ident[:])
                    nc.vector.tensor_copy(xt[:, dt, 128 * m:128 * (m + 1)],
                                          ps[:])
            # -- attention, per head group
            oT = acts.tile([128, DT, T], bf16, name=f"oT{slot_j}", tag="b512", bufs=2)
            for g in range(NHG):
                qkT = acts.tile([128, DT, T], bf16, name=f"qkT{slot_j}_{g}",
                                tag="b512", bufs=2)
                for half in range(2):   # 0: q block, 1: k block
                    wqkg = wstr.tile([128, DT, 384], bf16, tag="wqk",
                                     name=f"wqk{slot_j}_{g}_{half}", bufs=1)
                    nc.sync.dma_start(
                        out=wqkg[:],
                        in_=tiled(wd[f"wqk{l}"])[:, :,
                                                 768 * half + 384 * g:
                                                 768 * half + 384 * (g + 1)])
                    for jt in range(3):
                        ps = pmm.tile([128, T], f32, tag="mm",
                                      name=f"qk{slot_j}_{g}_{half}_{jt}")
                        for kk in range(DT):
                            nc.tensor.matmul(ps[:],
                                             wqkg[:, kk, 128 * jt:128 * (jt + 1)],
                                             xt[:, kk, :], start=(kk == 0),
                                             stop=(kk == DT - 1))
                        nc.vector.tensor_copy(qkT[:, 3 * half + jt, :], ps[:])
                vt = acts.tile([128, MT, HPG, HD + 1], bf16,
                               name=f"vt{slot_j}_{g}", tag="vt")
                nc.vector.memset(vt[:, :, :, HD:HD + 1], 1.0)
                wvg = wstr.tile([128, DT, 384], bf16, tag="wv",
                                name=f"wv{slot_j}_{g}", bufs=1)
                nc.sync.dma_start(out=wvg[:],
                                  in_=tiled(wd[f"wv{l}"])[:, :, 384 * g:384 * (g + 1)])
                for m in range(MT):
                    ps = pmm.tile([128, 384], f32, tag="mm",
                                  padded_shape=[128, 512],
                                  name=f"v{slot_j}_{g}_{m}")
                    for kk in range(DT):
                        nc.tensor.matmul(ps[:], xt[:, kk, 128 * m:128 * (m + 1)],
                                         wvg[:, kk, :], start=(kk == 0),
                                         stop=(kk == DT - 1))
                    nc.vector.tensor_copy(
                        vt[:, m, :, 0:HD],
                        ps[:].rearrange("p (h e) -> p h e", h=HPG))
                for h in range(HPG):
                    qh = qkT[64 * (h % 2):64 * (h % 2) + 64, h // 2, :]
                    eh = acts.tile([128, MT, T], bf16, tag="eh",
                                   name=f"eh{slot_j}_{g}_{h}", bufs=1)
                    for m in range(MT):
                        ps = pmm.tile([128, T], f32, tag="mm",
                                      name=f"sc{slot_j}_{g}_{h}_{m}")
                        kh = qkT[64 * (h % 2):64 * (h % 2) + 64, 3 + h // 2,
                                 128 * m:128 * (m + 1)]
                        nc.tensor.matmul(ps[:], kh, qh, start=True, stop=True)
                        nc.scalar.activation(eh[:, m, :], ps[:], AF.Exp,
                                             scale=float(1.0 / np.sqrt(HD)))
                    po = pmm.tile([HD + 1, T], f32, tag="mm",
                                  name=f"po{slot_j}_{g}_{h}")
                    for m in range(MT):
                        nc.tensor.matmul(po[:], vt[:, m, h, :], eh[:, m, :],
                                         start=(m == 0), stop=(m == MT - 1))
                    rcp = sm.tile([1, T], f32, tag="rcp", bufs=1,
                                  name=f"rcp{slot_j}_{g}_{h}")
                    nc.vector.reciprocal(rcp[:], po[HD:HD + 1, :])
                    psb = pmm.tile([64, T], f32, tag="mm",
                                   name=f"psb{slot_j}_{g}_{h}")
                    nc.tensor.matmul(psb[:], ones64[:], rcp[:], start=True,
                                     stop=True)
                    otmp = sm.tile([64, T], f32, tag="otmp", bufs=1,
                                   name=f"ot{slot_j}_{g}_{h}")
                    nc.vector.tensor_copy(otmp[:], po[0:HD, :])
                    nc.vector.tensor_mul(
                        oT[64 * (h % 2):64 * (h % 2) + 64, 3 * g + h // 2, :],
                        otmp[:], psb[:])

            def layernorm(xsl, dst):
                red = sm.tile([128, 1], f32, tag="red", name=f"red{slot_j}")
                nc.vector.tensor_reduce(red[:], xsl, mybir.AxisListType.X,
                                        mybir.AluOpType.add)
                nmean = sm.tile([128, 1], f32, tag="nmean",
                                name=f"nm{slot_j}")
                nc.vector.tensor_scalar_mul(nmean[:], red[:], -1.0 / D)
                nc.vector.tensor_scalar_add(xsl, xsl, nmean[:])
                sq = sm.tile([128, D], bf16, tag="sq", name=f"sq{slot_j}", bufs=1)
                vs = sm.tile([128, 1], f32, tag="vs", name=f"vs{slot_j}")
                nc.scalar.activation(sq[:], xsl, AF.Square, accum_out=vs[:])
                veps = sm.tile([128, 1], f32, tag="veps", name=f"ve{slot_j}")
                nc.vector.tensor_scalar(veps[:], vs[:], 1.0 / D, 1e-5,
                                        mybir.AluOpType.mult,
                                        mybir.AluOpType.add)
                rv = sm.tile([128, 1], f32, tag="rv", name=f"rv{slot_j}")
                nc.vector.reciprocal(rv[:], veps[:])
                rstd = sm.tile([128, 1], f32, tag="rstd", name=f"rs{slot_j}")
                nc.scalar.activation(rstd[:], rv[:], AF.Sqrt)
                nc.vector.tensor_scalar_mul(dst, xsl, rstd[:])

            # -- out-proj + residual -> LN1 -> x1
            xs = acts.tile([128, MT, D], f32, name=f"xs{slot_j}", tag="xs")
            for n in range(2):
                won = wstr.tile([128, DT, 384], bf16, tag="wo",
                                name=f"wo{slot_j}_{n}", bufs=1)
                nc.sync.dma_start(out=won[:],
                                  in_=tiled(wd[f"wo{l}"])[:, :, 384 * n:384 * (n + 1)])
                for m in range(MT):
                    ps = pmm.tile([128, 384], f32, tag="mm",
                                  padded_shape=[128, 512],
                                  name=f"op{slot_j}_{n}_{m}")
                    for kt in range(DT):
                        nc.tensor.matmul(ps[:], oT[:, kt, 128 * m:128 * (m + 1)],
                                         won[:, kt, :], start=(kt == 0),
                                         stop=(kt == DT - 1))
                    nc.vector.tensor_add(xs[:, m, 384 * n:384 * (n + 1)],
                                         ps[:], xin_s[:, m, 384 * n:384 * (n + 1)])
            x1 = acts.tile([128, MT, D], bf16, name=f"x1{slot_j}", tag="x1")
            for m in range(MT):
                layernorm(xs[:, m, :], x1[:, m, :])
            x1T = acts.tile([128, DT, T], bf16, name=f"x1T{slot_j}", tag="xtsh", bufs=1)
            for dt in range(DT):
                for m in range(MT):
                    ps = ptp.tile([128, 128], bf16, tag="tp",
                                  padded_shape=[128, 1024],
                                  name=f"x1t{slot_j}_{dt}_{m}")
                    nc.tensor.transpose(ps[:], x1[:, m, 128 * dt:128 * (dt + 1)],
                                        ident[:])
                    nc.vector.tensor_copy(x1T[:, dt, 128 * m:128 * (m + 1)],
                                          ps[:])
            # -- FFN in quarter blocks (6 f-tiles each)
            for blk in range(4):
                hgq = acts.tile([128, 6, T], bf16, tag="b512",
                                name=f"hgq{slot_j}_{blk}", bufs=2)
                for ft in range(6):
                    w1c = wstr.tile([128, DT, 128], bf16, tag="w1",
                                    name=f"w1{slot_j}_{blk}_{ft}", bufs=2)
                    fti = 6 * blk + ft
                    nc.sync.dma_start(
                        out=w1c[:],
                        in_=tiled(wd[f"w1{l}"])[:, :, 128 * fti:128 * (fti + 1)])
                    ps = pmm.tile([128, T], f32, tag="mm",
                                  name=f"f1{slot_j}_{blk}_{ft}")
                    for kk in range(DT):
                        nc.tensor.matmul(ps[:], w1c[:, kk, :], x1T[:, kk, :],
                                         start=(kk == 0), stop=(kk == DT - 1))
                    nc.scalar.activation(hgq[:, ft, :], ps[:], AF.Gelu)
                for n in range(2):
                    w2n = wstr.tile([128, 6, 384], bf16, tag="w2",
                                    name=f"w2{slot_j}_{blk}_{n}", bufs=1)
                    nc.sync.dma_start(
                        out=w2n[:],
                        in_=tiled(wd[f"w2{l}"])[:, 6 * blk:6 * blk + 6,
                                                384 * n:384 * (n + 1)])
                    for m in range(MT):
                        ps = pmm.tile([128, 384], f32, tag="mm",
                                      padded_shape=[128, 512],
                                      name=f"f2{slot_j}_{blk}_{n}_{m}")
                        for kt in range(6):
                            nc.tensor.matmul(ps[:],
                                             hgq[:, kt, 128 * m:128 * (m + 1)],
                                             w2n[:, kt, :], start=(kt == 0),
                                             stop=(kt == 5))
                        if blk == 0:
                            nc.vector.tensor_add(
                                xs[:, m, 384 * n:384 * (n + 1)], ps[:],
                                x1[:, m, 384 * n:384 * (n + 1)])
                        else:
                            nc.vector.tensor_add(
                                xs[:, m, 384 * n:384 * (n + 1)],
                                xs[:, m, 384 * n:384 * (n + 1)], ps[:])
            # -- LN2 -> x2 ; write out ; u vector
            x2 = acts.tile([128, MT, D], bf16, name=f"x2{slot_j}", tag="xin",
                           bufs=2)
            for m in range(MT):
                layernorm(xs[:, m, :], x2[:, m, :])
                if phase == 0:
                    nc.sync.dma_start(out=x0sc[k][:, m, :], in_=x2[:, m, :])
                else:
                    nc.sync.dma_start(out=tiled(xout_d)[:, 4 * k + m, :],
                                      in_=x2[:, m, :])
            u_sb = sm.tile([1, D], f32, tag="u_sb", name=f"usb{slot_j}", bufs=1)
            for n in range(2):
                psu = pmm.tile([1, 384], f32, tag="mm",
                               padded_shape=[128, 512],
                               name=f"u{slot_j}_{n}")
                for m in range(MT):
                    nc.tensor.matmul(psu[:], mfs[:, k, m:m + 1],
                                     x2[:, m, 384 * n:384 * (n + 1)],
                                     start=(m == 0), stop=(m == MT - 1))
                nc.vector.tensor_scalar_mul(u_sb[:, 384 * n:384 * (n + 1)],
                                            psu[:], icnt[:, k:k + 1])
            nc.sync.dma_start(out=agin[slot_j][:], in_=u_sb[:])
            nc.gpsimd.collective_compute(
                "AllGather", mybir.AluOpType.bypass,
                replica_groups=[list(range(N_CORES))],
                ins=[agin[slot_j][:].opt()], outs=[agout[slot_j][:].opt()])
            # -- gathered u -> banks (U1, AT) for this slot's real members
            ug = sm.tile([N_CORES, D], f32, tag="ug", name=f"ug{slot_j}", bufs=1)
            nc.sync.dma_start(out=ug[:], in_=agout[slot_j][:])
            upad = sm.tile([128, D], bf16, tag="upad", name=f"upad{slot_j}", bufs=1)
            nc.vector.memset(upad[:], 0.0)
            nc.vector.tensor_copy(upad[0:N_CORES, :], ug[:])
            umatT = sm.tile([128, DT, N_CORES], bf16, tag="umatT",
                            name=f"umT{slot_j}")
            for kk in range(DT):
                ps = ptp.tile([128, 128], bf16, tag="tp",
                              padded_shape=[128, 1024],
                              name=f"ut{slot_j}_{kk}")
                nc.tensor.transpose(ps[:], upad[:, 128 * kk:128 * (kk + 1)],
                                    ident[:])
                nc.vector.tensor_copy(umatT[:, kk, :], ps[:, 0:N_CORES])
            for n in range(4):
                w1t = wstr.tile([128, DT, 512], bf16, tag="wg1top",
                                name=f"wgt{slot_j}_{n}", bufs=1)
                nc.sync.dma_start(
                    out=w1t[:],
                    in_=tiled(wd[f"wg1{l}"])[:, 0:DT, 512 * n:512 * (n + 1)])
                ps = pmm.tile([N_CORES, 512], f32, tag="mm",
                              name=f"u1{slot_j}_{n}")
                for kk in range(DT):
                    nc.tensor.matmul(ps[:], umatT[:, kk, :], w1t[:, kk, :],
                                     start=(kk == 0), stop=(kk == DT - 1))
                u1tmp = sm.tile([S, 512], bf16, tag="u1tmp",
                                name=f"u1t{slot_j}_{n}")
                nc.vector.tensor_copy(u1tmp[:], ps[:])
                nc.sync.dma_start(out=U1slot[k][:, 512 * n:512 * (n + 1)],
                                  in_=u1tmp[:])
            for dt in range(DT):
                wat = wstr.tile([128, DT, 128], bf16, tag="wattn",
                                name=f"wat{slot_j}_{dt}", bufs=2)
                nc.sync.dma_start(
                    out=wat[:],
                    in_=tiled(wd[f"wattn{l}"])[:, :, 128 * dt:128 * (dt + 1)])
                ps = pmm.tile([128, N_CORES], f32, tag="mm",
                              padded_shape=[128, 512], name=f"at{slot_j}_{dt}")
                for kk in range(DT):
                    nc.tensor.matmul(ps[:], wat[:, kk, :], umatT[:, kk, :],
                                     start=(kk == 0), stop=(kk == DT - 1))
                nc.vector.tensor_copy(ATslot[k][:, dt, :], ps[:])

        def gate_code(l, b, wg1mb, wg2r):
            sid = int(sids[b])
            kb, ib = assign[b]
            u1row = sm.tile([1, U], bf16, tag="u1row", name=f"u1r{l}_{b}",
                            bufs=1)
            nc.sync.dma_start(out=u1row[:], in_=U1slot[kb][ib:ib + 1, :])
            pssc = pmm.tile([S, 1], f32, tag="mm", padded_shape=[128, 512],
                            name=f"gsc{l}_{b}")
            for kk in range(DT):
                nc.tensor.matmul(pssc[:], hT_bf[:, kk, :],
                                 ATslot[kb][:, kk, ib:ib + 1],
                                 start=(kk == 0), stop=(kk == DT - 1))
            sc_sb = sm.tile([128, 1], bf16, tag="scsb", name=f"gscs{l}_{b}")
            nc.vector.memset(sc_sb[:], 0.0)
            nc.vector.tensor_copy(sc_sb[0:S, :], pssc[:])
            pst = ptp.tile([128, 128], bf16, tag="tp", padded_shape=[128, 1024],
                           name=f"gst{l}_{b}")
            nc.tensor.transpose(pst[0:1, :], sc_sb[:], ident[:])
            srow = sm.tile([1, S], f32, tag="srow", name=f"gsr{l}_{b}")
            nc.vector.tensor_copy(srow[:], pst[0:1, 0:S])
            rmax = sm.tile([1, 1], f32, tag="rmax", name=f"grm{l}_{b}")
            nc.vector.tensor_reduce(rmax[:], srow[:], mybir.AxisListType.X,
                                    mybir.AluOpType.max)
            nmax = sm.tile([1, 1], f32, tag="nmax", name=f"gnm{l}_{b}")
            nc.vector.tensor_scalar_mul(nmax[:], rmax[:], -1.0)
            nc.vector.tensor_scalar_add(srow[:], srow[:], nmax[:])
            esum = sm.tile([1, 1], f32, tag="esum", name=f"ges{l}_{b}")
            erow = sm.tile([128, S], bf16, tag="erow", name=f"ger{l}_{b}")
            nc.vector.memset(erow[:], 0.0)
            nc.scalar.activation(erow[0:1, :], srow[:], AF.Exp,
                                 accum_out=esum[:])
            rinv = sm.tile([1, 1], f32, tag="rinv", name=f"gri{l}_{b}")
            nc.vector.reciprocal(rinv[:], esum[:])
            nc.vector.tensor_scalar_mul(erow[0:1, :], erow[0:1, :], rinv[:])
            pse = ptp.tile([128, 128], bf16, tag="tp", padded_shape=[128, 1024],
                           name=f"gpe{l}_{b}")
            nc.tensor.transpose(pse[0:8, :], erow[:], ident[:])
            esT = sm.tile([S, 1], bf16, tag="esT", name=f"geT{l}_{b}")
            nc.vector.tensor_copy(esT[:], pse[0:S, 0:1])
            ctxT = sm.tile([128, DT, 1], bf16, tag="ctxT", name=f"gcx{l}_{b}")
            for kk in range(DT):
                psc = pmm.tile([128, 1], f32, tag="mm", padded_shape=[128, 512],
                               name=f"gcp{l}_{b}_{kk}")
                nc.tensor.matmul(psc[:], h_bf[:, 128 * kk:128 * (kk + 1)],
                                 esT[:], start=True, stop=True)
                nc.vector.tensor_copy(ctxT[:, kk, :], psc[:])
            gT = sm.tile([128, UT, 1], bf16, tag="gT", name=f"ggT{l}_{b}")
            for n in range(4):
                psy = pmm.tile([1, 512], f32, tag="mm", name=f"gy{l}_{b}_{n}")
                for kk in range(DT):
                    nc.tensor.matmul(psy[:], hT_bf[:, kk, sid:sid + 1],
                                     wg1mb[:, kk, 512 * n:512 * (n + 1)],
                                     start=(kk == 0), stop=False)
                for kk in range(DT):
                    nc.tensor.matmul(psy[:], ctxT[:, kk, :],
                                     wg1mb[:, DT + kk, 512 * n:512 * (n + 1)],
                                     start=False, stop=False)
                nc.tensor.matmul(psy[:], ones128[0:1, 0:1],
                                 u1row[:, 512 * n:512 * (n + 1)],
                                 start=False, stop=True)
                gc = sm.tile([128, 512], bf16, tag="gc", name=f"ggc{l}_{b}_{n}")
                nc.vector.memset(gc[:], 0.0)
                nc.scalar.activation(gc[0:1, :], psy[:], AF.Gelu)
                for q in range(4):
                    ps3 = ptp.tile([128, 128], bf16, tag="tp",
                                   padded_shape=[128, 1024],
                                   name=f"ggt{l}_{b}_{n}_{q}")
                    nc.tensor.transpose(ps3[:], gc[:, 128 * q:128 * (q + 1)],
                                        ident[:])
                    nc.vector.tensor_copy(gT[:, 4 * n + q, :], ps3[:, 0:1])
            zrow = sm.tile([1, D], f32, tag="zrow", name=f"gz{l}_{b}", bufs=1)
            for n in range(2):
                psz = pmm.tile([1, 384], f32, tag="mm", padded_shape=[128, 512],
                               name=f"gzp{l}_{b}_{n}")
                for kk in range(UT):
                    nc.tensor.matmul(psz[:], gT[:, kk, :],
                                     wg2r[:, kk, 384 * n:384 * (n + 1)],
                                     start=(kk == 0), stop=(kk == UT - 1))
                nc.vector.tensor_copy(zrow[:, 384 * n:384 * (n + 1)], psz[:])
            red = sm.tile([1, 1], f32, tag="gred", name=f"gr{l}_{b}")
            nc.vector.tensor_reduce(red[:], zrow[:], mybir.AxisListType.X,
                                    mybir.AluOpType.add)
            nmean = sm.tile([1, 1], f32, tag="gnmean", name=f"gn{l}_{b}")
            nc.vector.tensor_scalar_mul(nmean[:], red[:], -1.0 / D)
            nc.vector.tensor_scalar_add(zrow[:], zrow[:], nmean[:])
            sq = sm.tile([1, D], bf16, tag="gsq", name=f"gq{l}_{b}", bufs=1)
            vs = sm.tile([1, 1], f32, tag="gvs", name=f"gv{l}_{b}")
            nc.scalar.activation(sq[:], zrow[:], AF.Square, accum_out=vs[:])
            veps = sm.tile([1, 1], f32, tag="gveps", name=f"gve{l}_{b}")
            nc.vector.tensor_scalar(veps[:], vs[:], 1.0 / D, 1e-5,
                                    mybir.AluOpType.mult, mybir.AluOpType.add)
            rv = sm.tile([1, 1], f32, tag="grv", name=f"grv{l}_{b}")
            nc.vector.reciprocal(rv[:], veps[:])
            rstd = sm.tile([1, 1], f32, tag="grstd", name=f"grs{l}_{b}")
            nc.scalar.activation(rstd[:], rv[:], AF.Sqrt)
            new_bf = sm.tile([1, D], bf16, tag="new_bf", name=f"gnb{l}_{b}", bufs=1)
            nc.vector.tensor_scalar_mul(new_bf[:], zrow[:], rstd[:])
            for n in range(2):
                psu = pmm.tile([S, 384], f32, tag="mm", padded_shape=[128, 512],
                               name=f"gsu{l}_{b}_{n}")
                nc.tensor.matmul(psu[:], onehr[0:1, b, :],
                                 new_bf[:, 384 * n:384 * (n + 1)],
                                 start=True, stop=True)
                nc.vector.tensor_add(h_spk[:, 384 * n:384 * (n + 1)],
                                     h_spk[:, 384 * n:384 * (n + 1)], psu[:])
            nc.vector.tensor_copy(h_bf[:], h_spk[:])
            for kk in range(DT):
                psv = pmm.tile([128, S], f32, tag="mm", padded_shape=[128, 512],
                               name=f"gsv{l}_{b}_{kk}")
                nc.tensor.matmul(psv[:], new_bf[:, 128 * kk:128 * (kk + 1)],
                                 onehr[0:1, b, :], start=True, stop=True)
                hTf = sm.tile([128, S], f32, tag="hTf", name=f"ghf{l}_{b}_{kk}")
                nc.vector.tensor_add(hTf[:], psv[:], hT_bf[:, kk, :])
                nc.vector.tensor_copy(hT_bf[:, kk, :], hTf[:])

        # ================= program =================
        for phase in range(L):
            l = phase
            wg1mb = wres.tile([128, 2 * DT, U], bf16, tag="wg1mb",
                              name=f"wg1mb{phase}")
            nc.sync.dma_start(out=wg1mb[:],
                              in_=tiled(wd[f"wg1{l}"])[:, DT:3 * DT, :])
            wg2r = wres.tile([128, UT, D], bf16, tag="wg2r",
                             name=f"wg2r{phase}")
            nc.sync.dma_start(out=wg2r[:], in_=tiled(wd[f"wg2{l}"]))
            nc.vector.memset(h_spk[:], 0.0)
            nc.vector.memset(h_bf[:], 0.0)
            nc.vector.memset(hT_bf[:], 0.0)
            for k, wave in enumerate(waves):
                slot_j = phase * len(waves) + k
                slot_code(phase, k, wave, slot_j)
                for b in groups[k]:
                    gate_code(l, b, wg1mb, wg2r)

    nc.compile()
    return nc


# -------------------------------------------------------------------- runner
class _Runner:
    """Compile once, execute via jitted shard_map binding _bass_exec_p."""

    def __init__(self, nc):
        import jax
        from jax.sharding import Mesh, PartitionSpec
        from jax.experimental.shard_map import shard_map
        from concourse import mybir
        from concourse.bass2jax import (_bass_exec_p, install_neuronx_cc_hook,
                                        partition_id_tensor)
        install_neuronx_cc_hook()
        self.nc = nc
        pid_name = (nc.partition_id_tensor.name
                    if nc.partition_id_tensor else None)
        in_names, out_names, out_avals = [], [], []
        for alloc in nc.m.functions[0].allocations:
            if not isinstance(alloc, mybir.MemoryLocationSet):
                continue
            name = alloc.memorylocations[0].name
            if alloc.kind == "ExternalInput":
                if name != pid_name:
                    in_names.append(name)
            elif alloc.kind == "ExternalOutput":
                out_names.append(name)
                shape = tuple(alloc.tensor_shape)
                dtype = mybir.dt.np(alloc.dtype)
                out_avals.append(jax.core.ShapedArray(shape, dtype))
        self.in_names = in_names
        self.out_names = out_names
        self.out_shapes = [tuple(a.shape) for a in out_avals]
        all_names = in_names + out_names
        if pid_name is not None:
            all_names = all_names + [pid_name]

        def _body(*args):
            operands = list(args)
            if pid_name is not None:
                operands.append(partition_id_tensor())
            outs = _bass_exec_p.bind(
                *operands, out_avals=tuple(out_avals),
                in_names=tuple(all_names), out_names=tuple(out_names),
                lowering_input_output_aliases=(),
                sim_require_finite=False, sim_require_nnan=False, nc=nc)
            return tuple(outs)

        devices = jax.devices()[:N_CORES]
        self.mesh = Mesh(np.asarray(devices), ("core",))
        self.pspec = PartitionSpec("core")
        nin = len(in_names) + len(out_names)
        self.fn = jax.jit(
            shard_map(_body, mesh=self.mesh,
                      in_specs=(self.pspec,) * nin,
                      out_specs=(self.pspec,) * len(out_names),
                      check_rep=False),
            keep_unused=True)
        self._concat_fn = None
        self.zero_outs = [
            self.put(np.zeros((N_CORES * s[0],) + s[1:], a.dtype))
            for s, a in zip(self.out_shapes, out_avals)]

    def put(self, arr):
        import jax
        from jax.sharding import NamedSharding
        return jax.device_put(arr, NamedSharding(self.mesh, self.pspec))

    def put_same(self, arr):
        import jax
        import jax.numpy as jnp
        from jax.sharding import NamedSharding, PartitionSpec
        rep = jax.device_put(arr, NamedSharding(self.mesh, PartitionSpec()))
        if self._concat_fn is None:
            self._concat_fn = jax.jit(
                lambda w: jnp.concatenate([w] * N_CORES, axis=0),
                out_shardings=NamedSharding(self.mesh, self.pspec))
        return self._concat_fn(rep)

    def run(self, dev_in):
        outs = self.fn(*[dev_in[n] for n in self.in_names], *self.zero_outs)
        for o in outs:
            o.block_until_ready()
        return outs


_STATE = {}


def _ln_host(x, s, b, eps=1e-5):
    m = x.mean(-1, keepdims=True)
    v = ((x - m) ** 2).mean(-1, keepdims=True)
    return (x - m) / np.sqrt(v + eps) * s + b


def _kernel_host(ids, sids, W):
    from scipy.special import erf

    def _gelu(x):
        return x * 0.5 * (1.0 + erf(x / np.sqrt(2.0)))

    def _softmax(x, axis=-1):
        x = x - x.max(axis=axis, keepdims=True)
        e = np.exp(x)
        return e / e.sum(axis=axis, keepdims=True)

    mask = (ids != 0)
    pos = (np.arange(T)[None, :] * mask.astype(np.int64))
    emb = W['tok_emb'][ids] + W['pos_emb'][pos]
    for l in range(L):
        h_spk = np.zeros((S, D), np.float32)
        out = np.empty_like(emb)
        for b in range(B):
            sid = int(sids[b])
            x = emb[b] + h_spk[sid] @ W['Wproj'][l]
            qkv = (x @ W['Wqkv'][l] + W['bqkv'][l]).reshape(T, 3, H, HD)
            q, k, v = qkv[:, 0], qkv[:, 1], qkv[:, 2]
            a = _softmax(np.einsum('thd,shd->hts', q, k) / np.sqrt(HD), -1)
            o = np.einsum('hts,shd->thd', a, v).reshape(T, D) @ W['Wout'][l] + W['bout'][l]
            x = _ln_host(x + o, W['ln1_s'][l], W['ln1_b'][l])
            hh = _gelu(x @ W['W1'][l] + W['b1'][l]) @ W['W2'][l] + W['b2'][l]
            x = _ln_host(x + hh, W['ln2_s'][l], W['ln2_b'][l])
            mf = mask[b].astype(np.float32)[:, None]
            u = (x * mf).sum(0) / max(mf.sum(), 1e-6)
            w = _softmax(h_spk @ (u @ W['Wattn'][l]))
            ctx = w @ h_spk
            mat = np.concatenate([u, h_spk[sid], ctx])
            new_h = _ln_host(_gelu(mat @ W['Wg1'][l] + W['bg1'][l]) @ W['Wg2'][l]
                             + W['bg2'][l], W['lng_s'][l], W['lng_b'][l])
            h_spk[sid] = h_spk[sid] + new_h
            out[b] = x
        emb = out
    return emb.astype(np.float32)


def kernel(**inputs):
    import ml_dtypes
    import time
    global LAST_EXEC_NS
    bf16 = ml_dtypes.bfloat16

    inp = {k: np.asarray(v) for k, v in inputs.items()}
    ids = inp['input_ids'].astype(np.int64)
    sids = inp['speaker_ids'].astype(np.int64)
    W = {k: inp[k].astype(np.float32) for k in inp
         if k not in ('input_ids', 'speaker_ids')}
    trivial = (all(not W[k].any() for k in
                   ('bqkv', 'bout', 'ln1_b', 'b1', 'b2', 'ln2_b',
                    'bg1', 'bg2', 'lng_b'))
               and (W['ln1_s'] == 1).all() and (W['ln2_s'] == 1).all()
               and (W['lng_s'] == 1).all())
    if not trivial:
        return _kernel_host(ids, sids, W)

    waves, groups, assign, prev = _schedule(sids)
    NW = len(waves)

    mask = (ids != 0)
    pos = (np.arange(T)[None, :] * mask.astype(np.int64))
    emb = W['tok_emb'][ids] + W['pos_emb'][pos]      # [B,T,D] f32

    key = ('nc', tuple(sids.tolist()))
    if key not in _STATE:
        nc = _build(sids.tolist(), waves, groups)
        _STATE[key] = _Runner(nc)
    runner = _STATE[key]

    # ---- per-core inputs
    turn_of = [[wave[i] if i < len(wave) else wave[0] for i in range(N_CORES)]
               for k, wave in enumerate(waves)]
    per_core = []
    mf01 = mask.astype(np.float32)                    # [B, T] 0/1
    icnt_b = 1.0 / np.clip(mask.sum(1).astype(np.float32), 1e-6, None)  # [B]
    for c in range(N_CORES):
        xemb = np.concatenate([emb[turn_of[k][c]] for k in range(NW)],
                              axis=0).astype(bf16)    # [NW*512, 768]
        mfs = np.empty((128, NW * MT), np.float32)
        icnt = np.empty((1, NW), np.float32)
        for k in range(NW):
            col = mf01[turn_of[k][c]]                 # [512]
            mfs[:, k * MT:(k + 1) * MT] = col.reshape(MT, 128).T
            icnt[0, k] = icnt_b[turn_of[k][c]]
        onehc = np.zeros((S, NW), np.float32)
        for k in range(NW):
            onehc[int(sids[turn_of[k][c]]), k] = 1.0
        per_core.append({'xemb': xemb, 'mfs': mfs.astype(bf16), 'icnt': icnt,
                         'onehc': onehc.astype(bf16)})
    onehr = np.zeros((B, S), np.float32)
    for b in range(B):
        onehr[b, int(sids[b])] = 1.0

    dev_in = {}
    for name in ('xemb', 'mfs', 'icnt', 'onehc'):
        dev_in[name] = runner.put(
            np.concatenate([pc[name] for pc in per_core], axis=0))
    dev_in['onehr'] = runner.put_same(onehr.reshape(1, -1).astype(bf16))
    if True:
        for l in range(L):
            Wqkv = W['Wqkv'][l]
            dev_in[f'wqk{l}'] = runner.put_same(
                np.ascontiguousarray(Wqkv[:, :2 * D]).astype(bf16))
            dev_in[f'wv{l}'] = runner.put_same(
                np.ascontiguousarray(Wqkv[:, 2 * D:]).astype(bf16))
            dev_in[f'wo{l}'] = runner.put_same(W['Wout'][l].astype(bf16))
            dev_in[f'w1{l}'] = runner.put_same(W['W1'][l].astype(bf16))
            dev_in[f'w2{l}'] = runner.put_same(W['W2'][l].astype(bf16))
            dev_in[f'wg1{l}'] = runner.put_same(W['Wg1'][l].astype(bf16))
            dev_in[f'wg2{l}'] = runner.put_same(W['Wg2'][l].astype(bf16))
            dev_in[f'wattn{l}'] = runner.put_same(W['Wattn'][l].astype(bf16))
            dev_in[f'wproj{l}'] = runner.put_same(W['Wproj'][l].astype(bf16))

    outs = runner.run(dev_in)
    xout = np.asarray(outs[runner.out_names.index('xout')]).reshape(
        N_CORES, NW * T, D)

    result = np.empty((B, T, D), np.float32)
    for b in range(B):
        k, c = assign[b]
        result[b] = xout[c, k * T:(k + 1) * T, :]

    if os.environ.get("KERNEL_PROFILE") == "1":
        # warm wall of the single full invocation, minus the RPC floor
        # measured on a trivial dispatch through the same path.
        import jax
        import jax.numpy as jnp
        from jax.sharding import NamedSharding
        walls = []
        for _ in range(4):
            t0 = time.perf_counter()
            runner.run(dev_in)
            walls.append(time.perf_counter() - t0)
        wall = min(walls)
        triv = jax.jit(lambda a: a + 1.0)
        zx = jax.device_put(np.zeros((N_CORES, 128), np.float32),
                            NamedSharding(runner.mesh, runner.pspec))
        triv(zx).block_until_ready()
        floors = []
        for _ in range(4):
            t0 = time.perf_counter()
            triv(zx).block_until_ready()
            floors.append(time.perf_counter() - t0)
        floor = min(floors)
        est = wall - floor
        print(f"[kernel] warm wall {wall*1e3:.2f} ms, rpc floor "
              f"{floor*1e3:.2f} ms, exec est {est*1e3:.2f} ms")
        LAST_EXEC_NS = int(max(est, 1e-6) * 1e9)
    return result


# revision 3
# speedup vs baseline: 1.1274x; 1.1274x over previous
"""Trainium2 single-invocation Bass kernel for nn_HSauteUnit.

The entire 2-layer x 16-turn speaker-memory recurrence runs in ONE
8-core SPMD NEFF invocation (the previous kernel used 7 invocations at
~81ms RPC floor each).  Structure:

- Runtime schedule from speaker_ids: per layer, turns fall into
  dependency "waves" (W(b) = 1 + max_{j<=prev(b)} W(j)); each wave is an
  encoder slot where up to 8 turns run data-parallel, one per core
  (idle cores duplicate a real turn; results ignored).
- Layer-phased: all layer-0 slots+gates, then layer-1 (keeps one
  layer's gate weights SBUF-resident).  Turn b is pinned to the same
  core in both phases so the layer-0 output stays core-local (DRAM
  scratch).
- Speaker-memory gates run REPLICATED on all 8 cores (identical math,
  so no state broadcast).  The only cross-core traffic is a small
  AllGather of per-turn utterance vectors after each slot.
- Gate matvecs are batched per-slot where linearity allows
  (U1 = u @ Wg1_top, AT = (u @ Wattn)^T for all of the slot's turns).

Matmuls in bf16 with f32 accumulation; LN / softmax / speaker state in
f32.  Zero biases and unit LN scales are folded (checked on the host,
full-host numpy fallback otherwise).
"""

import os
import numpy as np

B, T, D, H, F, L, V, P, S, U = 16, 512, 768, 12, 3072, 2, 32000, 512, 8, 2048
HD = D // H
N_CORES = 8
DT = D // 128     # 6
MT = T // 128     # 4
UT = U // 128     # 16
NHG = 2           # head groups
HPG = H // NHG    # 6 heads per group

LAST_EXEC_NS = -1
CHAIN_K = 5


# ------------------------------------------------------------------ schedule
def _schedule(sids):
    """Waves per layer + gate placement + turn->core assignment."""
    prev = [-1] * B
    last = {}
    for b in range(B):
        s = int(sids[b])
        prev[b] = last.get(s, -1)
        last[s] = b
    Wv = [0] * B
    for b in range(B):
        if prev[b] >= 0:
            Wv[b] = 1 + max(Wv[j] for j in range(prev[b] + 1))
    nw = max(Wv) + 1
    waves = [[b for b in range(B) if Wv[b] == k] for k in range(nw)]
    groups, placed = [], 0
    for k in range(nw):
        grp = []
        while placed < B and Wv[placed] <= k:
            grp.append(placed)
            placed += 1
        groups.append(grp)
    assert placed == B
    # turn -> (wave k, core i);  dummy cores replicate wave[0]
    assign = {}
    for k, wave in enumerate(waves):
        for i, b in enumerate(wave):
            assert i < N_CORES
            assign[b] = (k, i)
    return waves, groups, assign, prev


# ------------------------------------------------------------------- builder
def _build(sids, waves, groups):
    from contextlib import ExitStack
    import concourse.bass as bass
    import concourse.tile as tile
    from concourse import bacc, mybir
    from concourse.masks import make_identity

    f32 = mybir.dt.float32
    bf16 = mybir.dt.bfloat16
    AF = mybir.ActivationFunctionType
    NW = len(waves)

    nc = bacc.Bacc("TRN2", target_bir_lowering=False, debug=False,
                   num_devices=N_CORES)

    # ---- DRAM I/O
    xemb_d = nc.dram_tensor("xemb", [NW * T, D], bf16, kind="ExternalInput")
    mfs_d = nc.dram_tensor("mfs", [128, NW * MT], bf16, kind="ExternalInput")
    icnt_d = nc.dram_tensor("icnt", [1, NW], f32, kind="ExternalInput")
    onehc_d = nc.dram_tensor("onehc", [S, NW], bf16, kind="ExternalInput")
    onehr_d = nc.dram_tensor("onehr", [1, B * S], bf16, kind="ExternalInput")
    wd = {}
    for l in range(L):
        wd[f"wqk{l}"] = nc.dram_tensor(f"wqk{l}", [D, 2 * D], bf16, kind="ExternalInput")
        wd[f"wv{l}"] = nc.dram_tensor(f"wv{l}", [D, D], bf16, kind="ExternalInput")
        wd[f"wo{l}"] = nc.dram_tensor(f"wo{l}", [D, D], bf16, kind="ExternalInput")
        wd[f"w1{l}"] = nc.dram_tensor(f"w1{l}", [D, F], bf16, kind="ExternalInput")
        wd[f"w2{l}"] = nc.dram_tensor(f"w2{l}", [F, D], bf16, kind="ExternalInput")
        wd[f"wg1{l}"] = nc.dram_tensor(f"wg1{l}", [3 * D, U], bf16, kind="ExternalInput")
        wd[f"wg2{l}"] = nc.dram_tensor(f"wg2{l}", [U, D], bf16, kind="ExternalInput")
        wd[f"wattn{l}"] = nc.dram_tensor(f"wattn{l}", [D, D], bf16, kind="ExternalInput")
        wd[f"wproj{l}"] = nc.dram_tensor(f"wproj{l}", [D, D], bf16, kind="ExternalInput")
    xout_d = nc.dram_tensor("xout", [NW * T, D], bf16, kind="ExternalOutput")

    def tiled(h, p=128):
        return h.ap().rearrange("(a p) n -> p a n", p=p)

    with tile.TileContext(nc) as tc, ExitStack() as ctx:
        const = ctx.enter_context(tc.tile_pool(name="const", bufs=1))
        wres = ctx.enter_context(tc.tile_pool(name="wres", bufs=1))
        wstr = ctx.enter_context(tc.tile_pool(name="wstr", bufs=1))
        acts = ctx.enter_context(tc.tile_pool(name="acts", bufs=1))
        gst = ctx.enter_context(tc.tile_pool(name="gst", bufs=1))
        sm = ctx.enter_context(tc.tile_pool(name="sm", bufs=2))
        dram = ctx.enter_context(tc.tile_pool(name="dram", bufs=1, space="DRAM"))
        pmm = ctx.enter_context(tc.tile_pool(name="pmm", bufs=5, space="PSUM"))
        ptp = ctx.enter_context(tc.tile_pool(name="ptp", bufs=3, space="PSUM"))

        ident = const.tile([128, 128], bf16)
        make_identity(nc, ident[:])
        ones64 = const.tile([1, 64], f32)
        nc.vector.memset(ones64[:], 1.0)
        ones128 = const.tile([1, 128], bf16)
        nc.vector.memset(ones128[:], 1.0)

        mfs = const.tile([128, NW, MT], bf16, name="mfs")
        nc.sync.dma_start(out=mfs[:],
                          in_=mfs_d.ap().rearrange("p (s m) -> p s m", m=MT))
        icnt = const.tile([1, NW], f32, name="icnt")
        nc.sync.dma_start(out=icnt[:], in_=icnt_d.ap())
        onehc = const.tile([S, NW], bf16, name="onehc")
        nc.sync.dma_start(out=onehc[:], in_=onehc_d.ap())
        onehr = const.tile([1, B, S], bf16, name="onehr")
        nc.sync.dma_start(out=onehr[:],
                          in_=onehr_d.ap().rearrange("p (b s) -> p b s", s=S))

        # DRAM scratch + bounce buffers
        x0sc = [dram.tile([128, MT, D], bf16, tag=f"x0_{k}", name=f"x0_{k}")
                for k in range(NW)]
        agin = [dram.tile([1, D], f32, tag=f"agi{j}", name=f"agi{j}")
                for j in range(2 * NW)]
        agout = [dram.tile([N_CORES, D], f32, tag=f"ago{j}", name=f"ago{j}")
                 for j in range(2 * NW)]

        # persistent gate state / per-slot u-projection tiles
        h_spk = gst.tile([S, D], f32, name="h_spk")
        h_bf = gst.tile([S, D], bf16, name="h_bf")
        hT_bf = gst.tile([128, DT, S], bf16, name="hT_bf")
        U1slot = [dram.tile([S, U], bf16, tag=f"u1s{k}", name=f"u1s{k}")
                  for k in range(NW)]
        ATslot = [gst.tile([128, DT, N_CORES], bf16, tag=f"ats{k}",
                           name=f"ats{k}") for k in range(NW)]
        assign = {}
        for k, wave in enumerate(waves):
            for i, b in enumerate(wave):
                assign[b] = (k, i)

        def slot_code(phase, k, wave, slot_j):
            l = phase
            # -- xin = src + (h_spk[sid] @ Wproj) broadcast row
            hrowT = sm.tile([128, DT, 1], bf16, tag="hrowT",
                            name=f"hrowT{slot_j}")
            for kk in range(DT):
                ps = pmm.tile([128, 1], f32, tag="mm", padded_shape=[128, 512],
                              name=f"hr{slot_j}_{kk}")
                nc.tensor.matmul(ps[:], h_bf[:, 128 * kk:128 * (kk + 1)],
                                 onehc[:, k:k + 1], start=True, stop=True)
                nc.vector.tensor_copy(hrowT[:, kk, :], ps[:])
            c_bf = sm.tile([1, D], bf16, tag="c_bf", name=f"cbf{slot_j}", bufs=1)
            for n in range(2):
                wproj_s = wstr.tile([128, DT, 384], bf16, tag="wproj",
                                    name=f"wpj{slot_j}_{n}", bufs=1)
                nc.sync.dma_start(out=wproj_s[:],
                                  in_=tiled(wd[f"wproj{l}"])[:, :, 384 * n:384 * (n + 1)])
                ps = pmm.tile([1, 384], f32, tag="mm", padded_shape=[128, 512],
                              name=f"c{slot_j}_{n}")
                for kk in range(DT):
                    nc.tensor.matmul(ps[:], hrowT[:, kk, :], wproj_s[:, kk, :],
                                     start=(kk == 0), stop=(kk == DT - 1))
                nc.vector.tensor_copy(c_bf[:, 384 * n:384 * (n + 1)], ps[:])
            xin_s = acts.tile([128, MT, D], bf16, name=f"xin{slot_j}",
                              tag="xin", bufs=2)
            src = tiled(xemb_d)[:, 4 * k:4 * k + 4, :] if phase == 0 \
                else x0sc[k][:]
            nc.sync.dma_start(out=xin_s[:], in_=src)
            for n in range(2):
                psr = pmm.tile([128, 384], f32, tag="mm",
                               padded_shape=[128, 512], name=f"rep{slot_j}_{n}")
                nc.tensor.matmul(psr[:], ones128[:],
                                 c_bf[:, 384 * n:384 * (n + 1)],
                                 start=True, stop=True)
                for m in range(MT):
                    nc.vector.tensor_add(
                        xin_s[:, m, 384 * n:384 * (n + 1)],
                        xin_s[:, m, 384 * n:384 * (n + 1)], psr[:])
            # -- xt (feature-major)
            xt = acts.tile([128, DT, T], bf16, name=f"xt{slot_j}", tag="xtsh", bufs=1)
            for dt in range(DT):
                for m in range(MT):
                    ps = ptp.tile([128, 128], bf16, tag="tp",
                                  padded_shape=[128, 1024],
                                  name=f"xtt{slot_j}_{dt}_{m}")
                    nc.tensor.transpose(ps[:],
                                        xin_s[:, m, 128 * dt:128 * (dt + 1)],
                                        ident[:])
                    nc.vector.tensor_copy(xt[:, dt, 128 * m:128 * (m + 1)],
                                          ps[:])
            # -- attention, per head group
            oT = acts.tile([128, DT, T], bf16, name=f"oT{slot_j}", tag="b512", bufs=2)
            for g in range(NHG):
                qkT = acts.tile([128, DT, T], bf16, name=f"qkT{slot_j}_{g}",
                                tag="b512", bufs=2)
                for half in range(2):   # 0: q block, 1: k block
                    wqkg = wstr.tile([128, DT, 384], bf16, tag="wqk",
                                     name=f"wqk{slot_j}_{g}_{half}", bufs=1)
                    nc.sync.dma_start(
                        out=wqkg[:],
                        in_=tiled(wd[f"wqk{l}"])[:, :,
                                                 768 * half + 384 * g:
                                                 768 * half + 384 * (g + 1)])
                    for jt in range(3):
                        ps = pmm.tile([128, T], f32, tag="mm",
                                      name=f"qk{slot_j}_{g}_{half}_{jt}")
                        for kk in range(DT):
                            nc.tensor.matmul(ps[:],
                                             wqkg[:, kk, 128 * jt:128 * (jt + 1)],
                                             xt[:, kk, :], start=(kk == 0),
                                             stop=(kk == DT - 1))
                        nc.vector.tensor_copy(qkT[:, 3 * half + jt, :], ps[:])
                vt = acts.tile([128, MT, HPG, HD + 1], bf16,
                               name=f"vt{slot_j}_{g}", tag="vt")
                nc.vector.memset(vt[:, :, :, HD:HD + 1], 1.0)
                wvg = wstr.tile([128, DT, 384], bf16, tag="wv",
                                name=f"wv{slot_j}_{g}", bufs=1)
                nc.sync.dma_start(out=wvg[:],
                                  in_=tiled(wd[f"wv{l}"])[:, :, 384 * g:384 * (g + 1)])
                for m in range(MT):
                    ps = pmm.tile([128, 384], f32, tag="mm",
                                  padded_shape=[128, 512],
                                  name=f"v{slot_j}_{g}_{m}")
                    for kk in range(DT):
                        nc.tensor.matmul(ps[:], xt[:, kk, 128 * m:128 * (m + 1)],
                                         wvg[:, kk, :], start=(kk == 0),
                                         stop=(kk == DT - 1))
                    nc.vector.tensor_copy(
                        vt[:, m, :, 0:HD],
                        ps[:].rearrange("p (h e) -> p h e", h=HPG))
                for h in range(HPG):
                    qh = qkT[64 * (h % 2):64 * (h % 2) + 64, h // 2, :]
                    eh = acts.tile([128, MT, T], bf16, tag="eh",
                                   name=f"eh{slot_j}_{g}_{h}", bufs=2)
                    for m in range(MT):
                        ps = pmm.tile([128, T], f32, tag="mm",
                                      name=f"sc{slot_j}_{g}_{h}_{m}")
                        kh = qkT[64 * (h % 2):64 * (h % 2) + 64, 3 + h // 2,
                                 128 * m:128 * (m + 1)]
                        nc.tensor.matmul(ps[:], kh, qh, start=True, stop=True)
                        nc.scalar.activation(eh[:, m, :], ps[:], AF.Exp,
                                             scale=float(1.0 / np.sqrt(HD)))
                    po = pmm.tile([HD + 1, T], f32, tag="mm",
                                  name=f"po{slot_j}_{g}_{h}")
                    for m in range(MT):
                        nc.tensor.matmul(po[:], vt[:, m, h, :], eh[:, m, :],
                                         start=(m == 0), stop=(m == MT - 1))
                    rcp = sm.tile([1, T], f32, tag="rcp", bufs=1,
                                  name=f"rcp{slot_j}_{g}_{h}")
                    nc.vector.reciprocal(rcp[:], po[HD:HD + 1, :])
                    psb = pmm.tile([64, T], f32, tag="mm",
                                   name=f"psb{slot_j}_{g}_{h}")
                    nc.tensor.matmul(psb[:], ones64[:], rcp[:], start=True,
                                     stop=True)
                    otmp = sm.tile([64, T], f32, tag="otmp", bufs=1,
                                   name=f"ot{slot_j}_{g}_{h}")
                    nc.vector.tensor_copy(otmp[:], po[0:HD, :])
                    nc.vector.tensor_mul(
                        oT[64 * (h % 2):64 * (h % 2) + 64, 3 * g + h // 2, :],
                        otmp[:], psb[:])

            def layernorm(xsl, dst):
                red = sm.tile([128, 1], f32, tag="red", name=f"red{slot_j}")
                nc.vector.tensor_reduce(red[:], xsl, mybir.AxisListType.X,
                                        mybir.AluOpType.add)
                nmean = sm.tile([128, 1], f32, tag="nmean",
                                name=f"nm{slot_j}")
                nc.vector.tensor_scalar_mul(nmean[:], red[:], -1.0 / D)
                nc.vector.tensor_scalar_add(xsl, xsl, nmean[:])
                sq = sm.tile([128, D], bf16, tag="sq", name=f"sq{slot_j}", bufs=1)
                vs = sm.tile([128, 1], f32, tag="vs", name=f"vs{slot_j}")
                nc.scalar.activation(sq[:], xsl, AF.Square, accum_out=vs[:])
                veps = sm.tile([128, 1], f32, tag="veps", name=f"ve{slot_j}")
                nc.vector.tensor_scalar(veps[:], vs[:], 1.0 / D, 1e-5,
                                        mybir.AluOpType.mult,
                                        mybir.AluOpType.add)
                rv = sm.tile([128, 1], f32, tag="rv", name=f"rv{slot_j}")
                nc.vector.reciprocal(rv[:], veps[:])
                rstd = sm.tile([128, 1], f32, tag="rstd", name=f"rs{slot_j}")
                nc.scalar.activation(rstd[:], rv[:], AF.Sqrt)
                nc.vector.tensor_scalar_mul(dst, xsl, rstd[:])

            # -- out-proj + residual -> LN1 -> x1
            xs = acts.tile([128, MT, D], f32, name=f"xs{slot_j}", tag="xs")
            for n in range(2):
                won = wstr.tile([128, DT, 384], bf16, tag="wo",
                                name=f"wo{slot_j}_{n}", bufs=1)
                nc.sync.dma_start(out=won[:],
                                  in_=tiled(wd[f"wo{l}"])[:, :, 384 * n:384 * (n + 1)])
                for m in range(MT):
                    ps = pmm.tile([128, 384], f32, tag="mm",
                                  padded_shape=[128, 512],
                                  name=f"op{slot_j}_{n}_{m}")
                    for kt in range(DT):
                        nc.tensor.matmul(ps[:], oT[:, kt, 128 * m:128 * (m + 1)],
                                         won[:, kt, :], start=(kt == 0),
                                         stop=(kt == DT - 1))
                    nc.vector.tensor_add(xs[:, m, 384 * n:384 * (n + 1)],
                                         ps[:], xin_s[:, m, 384 * n:384 * (n + 1)])
            x1 = acts.tile([128, MT, D], bf16, name=f"x1{slot_j}", tag="x1")
            for m in range(MT):
                layernorm(xs[:, m, :], x1[:, m, :])
            x1T = acts.tile([128, DT, T], bf16, name=f"x1T{slot_j}", tag="xtsh", bufs=1)
            for dt in range(DT):
                for m in range(MT):
                    ps = ptp.tile([128, 128], bf16, tag="tp",
                                  padded_shape=[128, 1024],
                                  name=f"x1t{slot_j}_{dt}_{m}")
                    nc.tensor.transpose(ps[:], x1[:, m, 128 * dt:128 * (dt + 1)],
                                        ident[:])
                    nc.vector.tensor_copy(x1T[:, dt, 128 * m:128 * (m + 1)],
                                          ps[:])
            # -- FFN in quarter blocks (6 f-tiles each)
            for blk in range(4):
                hgq = acts.tile([128, 6, T], bf16, tag="b512",
                                name=f"hgq{slot_j}_{blk}", bufs=2)
                for ft in range(6):
                    w1c = wstr.tile([128, DT, 128], bf16, tag="w1",
                                    name=f"w1{slot_j}_{blk}_{ft}", bufs=2)
                    fti = 6 * blk + ft
                    nc.sync.dma_start(
                        out=w1c[:],
                        in_=tiled(wd[f"w1{l}"])[:, :, 128 * fti:128 * (fti + 1)])
                    ps = pmm.tile([128, T], f32, tag="mm",
                                  name=f"f1{slot_j}_{blk}_{ft}")
                    for kk in range(DT):
                        nc.tensor.matmul(ps[:], w1c[:, kk, :], x1T[:, kk, :],
                                         start=(kk == 0), stop=(kk == DT - 1))
                    nc.scalar.activation(hgq[:, ft, :], ps[:], AF.Gelu)
                for n in range(2):
                    w2n = wstr.tile([128, 6, 384], bf16, tag="w2",
                                    name=f"w2{slot_j}_{blk}_{n}", bufs=2)
                    nc.sync.dma_start(
                        out=w2n[:],
                        in_=tiled(wd[f"w2{l}"])[:, 6 * blk:6 * blk + 6,
                                                384 * n:384 * (n + 1)])
                    for m in range(MT):
                        ps = pmm.tile([128, 384], f32, tag="mm",
                                      padded_shape=[128, 512],
                                      name=f"f2{slot_j}_{blk}_{n}_{m}")
                        for kt in range(6):
                            nc.tensor.matmul(ps[:],
                                             hgq[:, kt, 128 * m:128 * (m + 1)],
                                             w2n[:, kt, :], start=(kt == 0),
                                             stop=(kt == 5))
                        if blk == 0:
                            nc.vector.tensor_add(
                                xs[:, m, 384 * n:384 * (n + 1)], ps[:],
                                x1[:, m, 384 * n:384 * (n + 1)])
                        else:
                            nc.vector.tensor_add(
                                xs[:, m, 384 * n:384 * (n + 1)],
                                xs[:, m, 384 * n:384 * (n + 1)], ps[:])
            # -- LN2 -> x2 ; write out ; u vector
            x2 = acts.tile([128, MT, D], bf16, name=f"x2{slot_j}", tag="xin",
                           bufs=2)
            for m in range(MT):
                layernorm(xs[:, m, :], x2[:, m, :])
                if phase == 0:
                    nc.sync.dma_start(out=x0sc[k][:, m, :], in_=x2[:, m, :])
                else:
                    nc.sync.dma_start(out=tiled(xout_d)[:, 4 * k + m, :],
                                      in_=x2[:, m, :])
            u_sb = sm.tile([1, D], f32, tag="u_sb", name=f"usb{slot_j}", bufs=1)
            for n in range(2):
                psu = pmm.tile([1, 384], f32, tag="mm",
                               padded_shape=[128, 512],
                               name=f"u{slot_j}_{n}")
                for m in range(MT):
                    nc.tensor.matmul(psu[:], mfs[:, k, m:m + 1],
                                     x2[:, m, 384 * n:384 * (n + 1)],
                                     start=(m == 0), stop=(m == MT - 1))
                nc.vector.tensor_scalar_mul(u_sb[:, 384 * n:384 * (n + 1)],
                                            psu[:], icnt[:, k:k + 1])
            nc.sync.dma_start(out=agin[slot_j][:], in_=u_sb[:])
            nc.gpsimd.collective_compute(
                "AllGather", mybir.AluOpType.bypass,
                replica_groups=[list(range(N_CORES))],
                ins=[agin[slot_j][:].opt()], outs=[agout[slot_j][:].opt()])
            # -- gathered u -> banks (U1, AT) for this slot's real members
            ug = sm.tile([N_CORES, D], f32, tag="ug", name=f"ug{slot_j}", bufs=1)
            nc.sync.dma_start(out=ug[:], in_=agout[slot_j][:])
            upad = sm.tile([128, D], bf16, tag="upad", name=f"upad{slot_j}", bufs=1)
            nc.vector.memset(upad[:], 0.0)
            nc.vector.tensor_copy(upad[0:N_CORES, :], ug[:])
            umatT = sm.tile([128, DT, N_CORES], bf16, tag="umatT",
                            name=f"umT{slot_j}")
            for kk in range(DT):
                ps = ptp.tile([128, 128], bf16, tag="tp",
                              padded_shape=[128, 1024],
                              name=f"ut{slot_j}_{kk}")
                nc.tensor.transpose(ps[:], upad[:, 128 * kk:128 * (kk + 1)],
                                    ident[:])
                nc.vector.tensor_copy(umatT[:, kk, :], ps[:, 0:N_CORES])
            for n in range(4):
                w1t = wstr.tile([128, DT, 512], bf16, tag="wg1top",
                                name=f"wgt{slot_j}_{n}", bufs=1)
                nc.sync.dma_start(
                    out=w1t[:],
                    in_=tiled(wd[f"wg1{l}"])[:, 0:DT, 512 * n:512 * (n + 1)])
                ps = pmm.tile([N_CORES, 512], f32, tag="mm",
                              name=f"u1{slot_j}_{n}")
                for kk in range(DT):
                    nc.tensor.matmul(ps[:], umatT[:, kk, :], w1t[:, kk, :],
                                     start=(kk == 0), stop=(kk == DT - 1))
                u1tmp = sm.tile([S, 512], bf16, tag="u1tmp",
                                name=f"u1t{slot_j}_{n}", bufs=1)
                nc.vector.tensor_copy(u1tmp[:], ps[:])
                nc.sync.dma_start(out=U1slot[k][:, 512 * n:512 * (n + 1)],
                                  in_=u1tmp[:])
            for dt in range(DT):
                wat = wstr.tile([128, DT, 128], bf16, tag="wattn",
                                name=f"wat{slot_j}_{dt}", bufs=2)
                nc.sync.dma_start(
                    out=wat[:],
                    in_=tiled(wd[f"wattn{l}"])[:, :, 128 * dt:128 * (dt + 1)])
                ps = pmm.tile([128, N_CORES], f32, tag="mm",
                              padded_shape=[128, 512], name=f"at{slot_j}_{dt}")
                for kk in range(DT):
                    nc.tensor.matmul(ps[:], wat[:, kk, :], umatT[:, kk, :],
                                     start=(kk == 0), stop=(kk == DT - 1))
                nc.vector.tensor_copy(ATslot[k][:, dt, :], ps[:])

        def gate_code(l, b, wg1mb, wg2r):
            sid = int(sids[b])
            kb, ib = assign[b]
            u1row = sm.tile([1, U], bf16, tag="u1row", name=f"u1r{l}_{b}",
                            bufs=1)
            nc.sync.dma_start(out=u1row[:], in_=U1slot[kb][ib:ib + 1, :])
            pssc = pmm.tile([S, 1], f32, tag="mm", padded_shape=[128, 512],
                            name=f"gsc{l}_{b}")
            for kk in range(DT):
                nc.tensor.matmul(pssc[:], hT_bf[:, kk, :],
                                 ATslot[kb][:, kk, ib:ib + 1],
                                 start=(kk == 0), stop=(kk == DT - 1))
            sc_sb = sm.tile([128, 1], bf16, tag="scsb", name=f"gscs{l}_{b}")
            nc.vector.memset(sc_sb[:], 0.0)
            nc.vector.tensor_copy(sc_sb[0:S, :], pssc[:])
            pst = ptp.tile([128, 128], bf16, tag="tp", padded_shape=[128, 1024],
                           name=f"gst{l}_{b}")
            nc.tensor.transpose(pst[0:1, :], sc_sb[:], ident[:])
            srow = sm.tile([1, S], f32, tag="srow", name=f"gsr{l}_{b}")
            nc.vector.tensor_copy(srow[:], pst[0:1, 0:S])
            rmax = sm.tile([1, 1], f32, tag="rmax", name=f"grm{l}_{b}")
            nc.vector.tensor_reduce(rmax[:], srow[:], mybir.AxisListType.X,
                                    mybir.AluOpType.max)
            nmax = sm.tile([1, 1], f32, tag="nmax", name=f"gnm{l}_{b}")
            nc.vector.tensor_scalar_mul(nmax[:], rmax[:], -1.0)
            nc.vector.tensor_scalar_add(srow[:], srow[:], nmax[:])
            esum = sm.tile([1, 1], f32, tag="esum", name=f"ges{l}_{b}")
            erow = sm.tile([128, S], bf16, tag="erow", name=f"ger{l}_{b}")
            nc.vector.memset(erow[:], 0.0)
            nc.scalar.activation(erow[0:1, :], srow[:], AF.Exp,
                                 accum_out=esum[:])
            rinv = sm.tile([1, 1], f32, tag="rinv", name=f"gri{l}_{b}")
            nc.vector.reciprocal(rinv[:], esum[:])
            nc.vector.tensor_scalar_mul(erow[0:1, :], erow[0:1, :], rinv[:])
            pse = ptp.tile([128, 128], bf16, tag="tp", padded_shape=[128, 1024],
                           name=f"gpe{l}_{b}")
            nc.tensor.transpose(pse[0:8, :], erow[:], ident[:])
            esT = sm.tile([S, 1], bf16, tag="esT", name=f"geT{l}_{b}")
            nc.vector.tensor_copy(esT[:], pse[0:S, 0:1])
            ctxT = sm.tile([128, DT, 1], bf16, tag="ctxT", name=f"gcx{l}_{b}")
            for kk in range(DT):
                psc = pmm.tile([128, 1], f32, tag="mm", padded_shape=[128, 512],
                               name=f"gcp{l}_{b}_{kk}")
                nc.tensor.matmul(psc[:], h_bf[:, 128 * kk:128 * (kk + 1)],
                                 esT[:], start=True, stop=True)
                nc.vector.tensor_copy(ctxT[:, kk, :], psc[:])
            gT = sm.tile([128, UT, 1], bf16, tag="gT", name=f"ggT{l}_{b}")
            for n in range(4):
                psy = pmm.tile([1, 512], f32, tag="mm", name=f"gy{l}_{b}_{n}")
                for kk in range(DT):
                    nc.tensor.matmul(psy[:], hT_bf[:, kk, sid:sid + 1],
                                     wg1mb[:, kk, 512 * n:512 * (n + 1)],
                                     start=(kk == 0), stop=False)
                for kk in range(DT):
                    nc.tensor.matmul(psy[:], ctxT[:, kk, :],
                                     wg1mb[:, DT + kk, 512 * n:512 * (n + 1)],
                                     start=False, stop=False)
                nc.tensor.matmul(psy[:], ones128[0:1, 0:1],
                                 u1row[:, 512 * n:512 * (n + 1)],
                                 start=False, stop=True)
                gc = sm.tile([128, 512], bf16, tag="gc", name=f"ggc{l}_{b}_{n}")
                nc.vector.memset(gc[:], 0.0)
                nc.scalar.activation(gc[0:1, :], psy[:], AF.Gelu)
                for q in range(4):
                    ps3 = ptp.tile([128, 128], bf16, tag="tp",
                                   padded_shape=[128, 1024],
                                   name=f"ggt{l}_{b}_{n}_{q}")
                    nc.tensor.transpose(ps3[:], gc[:, 128 * q:128 * (q + 1)],
                                        ident[:])
                    nc.vector.tensor_copy(gT[:, 4 * n + q, :], ps3[:, 0:1])
            zrow = sm.tile([1, D], f32, tag="zrow", name=f"gz{l}_{b}", bufs=1)
            for n in range(2):
                psz = pmm.tile([1, 384], f32, tag="mm", padded_shape=[128, 512],
                               name=f"gzp{l}_{b}_{n}")
                for kk in range(UT):
                    nc.tensor.matmul(psz[:], gT[:, kk, :],
                                     wg2r[:, kk, 384 * n:384 * (n + 1)],
                                     start=(kk == 0), stop=(kk == UT - 1))
                nc.vector.tensor_copy(zrow[:, 384 * n:384 * (n + 1)], psz[:])
            red = sm.tile([1, 1], f32, tag="gred", name=f"gr{l}_{b}")
            nc.vector.tensor_reduce(red[:], zrow[:], mybir.AxisListType.X,
                                    mybir.AluOpType.add)
            nmean = sm.tile([1, 1], f32, tag="gnmean", name=f"gn{l}_{b}")
            nc.vector.tensor_scalar_mul(nmean[:], red[:], -1.0 / D)
            nc.vector.tensor_scalar_add(zrow[:], zrow[:], nmean[:])
            sq = sm.tile([1, D], bf16, tag="gsq", name=f"gq{l}_{b}", bufs=1)
            vs = sm.tile([1, 1], f32, tag="gvs", name=f"gv{l}_{b}")
            nc.scalar.activation(sq[:], zrow[:], AF.Square, accum_out=vs[:])
            veps = sm.tile([1, 1], f32, tag="gveps", name=f"gve{l}_{b}")
            nc.vector.tensor_scalar(veps[:], vs[:], 1.0 / D, 1e-5,
                                    mybir.AluOpType.mult, mybir.AluOpType.add)
            rv = sm.tile([1, 1], f32, tag="grv", name=f"grv{l}_{b}")
            nc.vector.reciprocal(rv[:], veps[:])
            rstd = sm.tile([1, 1], f32, tag="grstd", name=f"grs{l}_{b}")
            nc.scalar.activation(rstd[:], rv[:], AF.Sqrt)
            new_bf = sm.tile([1, D], bf16, tag="new_bf", name=f"gnb{l}_{b}", bufs=1)
            nc.vector.tensor_scalar_mul(new_bf[:], zrow[:], rstd[:])
            for n in range(2):
                psu = pmm.tile([S, 384], f32, tag="mm", padded_shape=[128, 512],
                               name=f"gsu{l}_{b}_{n}")
                nc.tensor.matmul(psu[:], onehr[0:1, b, :],
                                 new_bf[:, 384 * n:384 * (n + 1)],
                                 start=True, stop=True)
                nc.vector.tensor_add(h_spk[:, 384 * n:384 * (n + 1)],
                                     h_spk[:, 384 * n:384 * (n + 1)], psu[:])
            nc.vector.tensor_copy(h_bf[:], h_spk[:])
            for kk in range(DT):
                psv = pmm.tile([128, S], f32, tag="mm", padded_shape=[128, 512],
                               name=f"gsv{l}_{b}_{kk}")
                nc.tensor.matmul(psv[:], new_bf[:, 128 * kk:128 * (kk + 1)],
                                 onehr[0:1, b, :], start=True, stop=True)
                hTf = sm.tile([128, S], f32, tag="hTf", name=f"ghf{l}_{b}_{kk}")
                nc.vector.tensor_add(hTf[:], psv[:], hT_bf[:, kk, :])
                nc.vector.tensor_copy(hT_bf[:, kk, :], hTf[:])

        # ================= program =================
        for phase in range(L):
            l = phase
            wg1mb = wres.tile([128, 2 * DT, U], bf16, tag="wg1mb",
                              name=f"wg1mb{phase}")
            nc.sync.dma_start(out=wg1mb[:],
                              in_=tiled(wd[f"wg1{l}"])[:, DT:3 * DT, :])
            wg2r = wres.tile([128, UT, D], bf16, tag="wg2r",
                             name=f"wg2r{phase}")
            nc.sync.dma_start(out=wg2r[:], in_=tiled(wd[f"wg2{l}"]))
            nc.vector.memset(h_spk[:], 0.0)
            nc.vector.memset(h_bf[:], 0.0)
            nc.vector.memset(hT_bf[:], 0.0)
            for k, wave in enumerate(waves):
                slot_j = phase * len(waves) + k
                slot_code(phase, k, wave, slot_j)
                for b in groups[k]:
                    gate_code(l, b, wg1mb, wg2r)

    nc.compile()
    return nc


# -------------------------------------------------------------------- runner
class _Runner:
    """Compile once, execute via jitted shard_map binding _bass_exec_p."""

    def __init__(self, nc):
        import jax
        from jax.sharding import Mesh, PartitionSpec
        from jax.experimental.shard_map import shard_map
        from concourse import mybir
        from concourse.bass2jax import (_bass_exec_p, install_neuronx_cc_hook,
                                        partition_id_tensor)
        install_neuronx_cc_hook()
        self.nc = nc
        pid_name = (nc.partition_id_tensor.name
                    if nc.partition_id_tensor else None)
        in_names, out_names, out_avals = [], [], []
        for alloc in nc.m.functions[0].allocations:
            if not isinstance(alloc, mybir.MemoryLocationSet):
                continue
            name = alloc.memorylocations[0].name
            if alloc.kind == "ExternalInput":
                if name != pid_name:
                    in_names.append(name)
            elif alloc.kind == "ExternalOutput":
                out_names.append(name)
                shape = tuple(alloc.tensor_shape)
                dtype = mybir.dt.np(alloc.dtype)
                out_avals.append(jax.core.ShapedArray(shape, dtype))
        self.in_names = in_names
        self.out_names = out_names
        self.out_shapes = [tuple(a.shape) for a in out_avals]
        all_names = in_names + out_names
        if pid_name is not None:
            all_names = all_names + [pid_name]

        def _exec_once(operands):
            ops = list(operands)
            if pid_name is not None:
                ops.append(partition_id_tensor())
            return _bass_exec_p.bind(
                *ops, out_avals=tuple(out_avals),
                in_names=tuple(all_names), out_names=tuple(out_names),
                lowering_input_output_aliases=(),
                sim_require_finite=False, sim_require_nnan=False, nc=nc)

        def _body(*args):
            return tuple(_exec_once(args))

        xemb_idx = in_names.index('xemb')

        def _body_chain(*args):
            import jax.numpy as jnp
            outs = _exec_once(args)
            for _ in range(CHAIN_K - 1):
                # serialize repeats: inject a zero-valued data dependency on
                # the previous output into one input
                dep = (outs[0][0, 0] * 0).astype(args[xemb_idx].dtype)
                args = list(args)
                args[xemb_idx] = args[xemb_idx] + dep
                outs = _exec_once(args)
            return tuple(outs)

        self._mk_chain = lambda: jax.jit(
            shard_map(_body_chain, mesh=self.mesh,
                      in_specs=(self.pspec,) * nin,
                      out_specs=(self.pspec,) * len(out_names),
                      check_rep=False),
            keep_unused=True)

        devices = jax.devices()[:N_CORES]
        self.mesh = Mesh(np.asarray(devices), ("core",))
        self.pspec = PartitionSpec("core")
        nin = len(in_names) + len(out_names)
        self.fn = jax.jit(
            shard_map(_body, mesh=self.mesh,
                      in_specs=(self.pspec,) * nin,
                      out_specs=(self.pspec,) * len(out_names),
                      check_rep=False),
            keep_unused=True)
        self._concat_fn = None
        self.zero_outs = [
            self.put(np.zeros((N_CORES * s[0],) + s[1:], a.dtype))
            for s, a in zip(self.out_shapes, out_avals)]

    def put(self, arr):
        import jax
        from jax.sharding import NamedSharding
        return jax.device_put(arr, NamedSharding(self.mesh, self.pspec))

    def put_same(self, arr):
        import jax
        import jax.numpy as jnp
        from jax.sharding import NamedSharding, PartitionSpec
        rep = jax.device_put(arr, NamedSharding(self.mesh, PartitionSpec()))
        if self._concat_fn is None:
            self._concat_fn = jax.jit(
                lambda w: jnp.concatenate([w] * N_CORES, axis=0),
                out_shardings=NamedSharding(self.mesh, self.pspec))
        return self._concat_fn(rep)

    def run(self, dev_in):
        outs = self.fn(*[dev_in[n] for n in self.in_names], *self.zero_outs)
        for o in outs:
            o.block_until_ready()
        return outs


_STATE = {}


def _ln_host(x, s, b, eps=1e-5):
    m = x.mean(-1, keepdims=True)
    v = ((x - m) ** 2).mean(-1, keepdims=True)
    return (x - m) / np.sqrt(v + eps) * s + b


def _kernel_host(ids, sids, W):
    from scipy.special import erf

    def _gelu(x):
        return x * 0.5 * (1.0 + erf(x / np.sqrt(2.0)))

    def _softmax(x, axis=-1):
        x = x - x.max(axis=axis, keepdims=True)
        e = np.exp(x)
        return e / e.sum(axis=axis, keepdims=True)

    mask = (ids != 0)
    pos = (np.arange(T)[None, :] * mask.astype(np.int64))
    emb = W['tok_emb'][ids] + W['pos_emb'][pos]
    for l in range(L):
        h_spk = np.zeros((S, D), np.float32)
        out = np.empty_like(emb)
        for b in range(B):
            sid = int(sids[b])
            x = emb[b] + h_spk[sid] @ W['Wproj'][l]
            qkv = (x @ W['Wqkv'][l] + W['bqkv'][l]).reshape(T, 3, H, HD)
            q, k, v = qkv[:, 0], qkv[:, 1], qkv[:, 2]
            a = _softmax(np.einsum('thd,shd->hts', q, k) / np.sqrt(HD), -1)
            o = np.einsum('hts,shd->thd', a, v).reshape(T, D) @ W['Wout'][l] + W['bout'][l]
            x = _ln_host(x + o, W['ln1_s'][l], W['ln1_b'][l])
            hh = _gelu(x @ W['W1'][l] + W['b1'][l]) @ W['W2'][l] + W['b2'][l]
            x = _ln_host(x + hh, W['ln2_s'][l], W['ln2_b'][l])
            mf = mask[b].astype(np.float32)[:, None]
            u = (x * mf).sum(0) / max(mf.sum(), 1e-6)
            w = _softmax(h_spk @ (u @ W['Wattn'][l]))
            ctx = w @ h_spk
            mat = np.concatenate([u, h_spk[sid], ctx])
            new_h = _ln_host(_gelu(mat @ W['Wg1'][l] + W['bg1'][l]) @ W['Wg2'][l]
                             + W['bg2'][l], W['lng_s'][l], W['lng_b'][l])
            h_spk[sid] = h_spk[sid] + new_h
            out[b] = x
        emb = out
    return emb.astype(np.float32)


def kernel(**inputs):
    import ml_dtypes
    import time
    global LAST_EXEC_NS
    bf16 = ml_dtypes.bfloat16

    inp = {k: np.asarray(v) for k, v in inputs.items()}
    ids = inp['input_ids'].astype(np.int64)
    sids = inp['speaker_ids'].astype(np.int64)
    W = {k: inp[k].astype(np.float32) for k in inp
         if k not in ('input_ids', 'speaker_ids')}
    trivial = (all(not W[k].any() for k in
                   ('bqkv', 'bout', 'ln1_b', 'b1', 'b2', 'ln2_b',
                    'bg1', 'bg2', 'lng_b'))
               and (W['ln1_s'] == 1).all() and (W['ln2_s'] == 1).all()
               and (W['lng_s'] == 1).all())
    if not trivial:
        return _kernel_host(ids, sids, W)

    waves, groups, assign, prev = _schedule(sids)
    NW = len(waves)

    mask = (ids != 0)
    pos = (np.arange(T)[None, :] * mask.astype(np.int64))
    emb = W['tok_emb'][ids] + W['pos_emb'][pos]      # [B,T,D] f32

    key = ('nc', tuple(sids.tolist()))
    if key not in _STATE:
        nc = _build(sids.tolist(), waves, groups)
        _STATE[key] = _Runner(nc)
    runner = _STATE[key]

    # ---- per-core inputs
    turn_of = [[wave[i] if i < len(wave) else wave[0] for i in range(N_CORES)]
               for k, wave in enumerate(waves)]
    per_core = []
    mf01 = mask.astype(np.float32)                    # [B, T] 0/1
    icnt_b = 1.0 / np.clip(mask.sum(1).astype(np.float32), 1e-6, None)  # [B]
    for c in range(N_CORES):
        xemb = np.concatenate([emb[turn_of[k][c]] for k in range(NW)],
                              axis=0).astype(bf16)    # [NW*512, 768]
        mfs = np.empty((128, NW * MT), np.float32)
        icnt = np.empty((1, NW), np.float32)
        for k in range(NW):
            col = mf01[turn_of[k][c]]                 # [512]
            mfs[:, k * MT:(k + 1) * MT] = col.reshape(MT, 128).T
            icnt[0, k] = icnt_b[turn_of[k][c]]
        onehc = np.zeros((S, NW), np.float32)
        for k in range(NW):
            onehc[int(sids[turn_of[k][c]]), k] = 1.0
        per_core.append({'xemb': xemb, 'mfs': mfs.astype(bf16), 'icnt': icnt,
                         'onehc': onehc.astype(bf16)})
    onehr = np.zeros((B, S), np.float32)
    for b in range(B):
        onehr[b, int(sids[b])] = 1.0

    dev_in = {}
    for name in ('xemb', 'mfs', 'icnt', 'onehc'):
        dev_in[name] = runner.put(
            np.concatenate([pc[name] for pc in per_core], axis=0))
    dev_in['onehr'] = runner.put_same(onehr.reshape(1, -1).astype(bf16))
    if True:
        for l in range(L):
            Wqkv = W['Wqkv'][l]
            dev_in[f'wqk{l}'] = runner.put_same(
                np.ascontiguousarray(Wqkv[:, :2 * D]).astype(bf16))
            dev_in[f'wv{l}'] = runner.put_same(
                np.ascontiguousarray(Wqkv[:, 2 * D:]).astype(bf16))
            dev_in[f'wo{l}'] = runner.put_same(W['Wout'][l].astype(bf16))
            dev_in[f'w1{l}'] = runner.put_same(W['W1'][l].astype(bf16))
            dev_in[f'w2{l}'] = runner.put_same(W['W2'][l].astype(bf16))
            dev_in[f'wg1{l}'] = runner.put_same(W['Wg1'][l].astype(bf16))
            dev_in[f'wg2{l}'] = runner.put_same(W['Wg2'][l].astype(bf16))
            dev_in[f'wattn{l}'] = runner.put_same(W['Wattn'][l].astype(bf16))
            dev_in[f'wproj{l}'] = runner.put_same(W['Wproj'][l].astype(bf16))

    outs = runner.run(dev_in)
    xout = np.asarray(outs[runner.out_names.index('xout')]).reshape(
        N_CORES, NW * T, D)

    result = np.empty((B, T, D), np.float32)
    for b in range(B):
        k, c = assign[b]
        result[b] = xout[c, k * T:(k + 1) * T, :]

    if os.environ.get("KERNEL_PROFILE") == "1":
        import jax
        from jax.sharding import NamedSharding
        est = None
        if os.environ.get("KERNEL_CHAIN") == "1":
            try:
                # k chained executions in ONE dispatch: the RPC floor
                # cancels exactly in the slope.
                fn_chain = runner._mk_chain()
                args = [dev_in[n] for n in runner.in_names] + runner.zero_outs
                outs = fn_chain(*args)
                for o in outs:
                    o.block_until_ready()
                w1s, wks = [], []
                for _ in range(4):
                    t0 = time.perf_counter()
                    runner.run(dev_in)
                    w1s.append(time.perf_counter() - t0)
                    t0 = time.perf_counter()
                    outs = fn_chain(*args)
                    for o in outs:
                        o.block_until_ready()
                    wks.append(time.perf_counter() - t0)
                if min(wks) > min(w1s):
                    est = (min(wks) - min(w1s)) / (CHAIN_K - 1)
                    print(f"[kernel] chain slope est {est*1e3:.2f} ms")
            except Exception as e:
                print(f"[kernel] chain profiling failed: {e}")
        if est is None:
            # subtract the RPC floor (trivial dispatch) from the warm wall,
            # interleaving samples so drift cancels
            triv = jax.jit(lambda a: a + 1.0)
            zx = jax.device_put(np.zeros((N_CORES, 128), np.float32),
                                NamedSharding(runner.mesh, runner.pspec))
            triv(zx).block_until_ready()
            walls, floors = [], []
            for _ in range(6):
                t0 = time.perf_counter()
                runner.run(dev_in)
                walls.append(time.perf_counter() - t0)
                t0 = time.perf_counter()
                triv(zx).block_until_ready()
                floors.append(time.perf_counter() - t0)
            est = max(min(walls) - min(floors), 1e-6)
            print(f"[kernel] warm wall {min(walls)*1e3:.2f} ms, floor "
                  f"{min(floors)*1e3:.2f} ms, exec est {est*1e3:.2f} ms")
        LAST_EXEC_NS = int(est * 1e9)
    return result
